# revision 3
# baseline (speedup 1.0000x reference)
"""Trainium2 Bass kernel for nn_Discriminator_15668040696127.

Computes:
    q, a, d = samples[:, 0], samples[:, 1], samples[:, 2]        # [B, D]
    cos1 = <q,d> / max(||q||*||d||, 1e-6)                         # [B]
    cos2 = <a,d> / max(||a||*||d||, 1e-6)                         # [B]
    score = cos1 @ D_v1 + cos2 @ D_v2                             # scalar
    out = BCE_with_logits(score, labels[0])                       # scalar

Sharding: data-parallel over B across 8 NeuronCores (1024 samples each).
Each core computes a partial score; an on-device AllReduce sums them and
every core evaluates the (scalar) BCE; the host reads core 0's output.
"""

import os
import sys

import numpy as np

for _p in ("/opt/trn_rl_repo", "/root/.axon_site/_ro/trn_rl_repo"):
    if os.path.isdir(_p) and _p not in sys.path:
        sys.path.append(_p)

import concourse.bass as bass
import concourse.bacc as bacc
import concourse.mybir as mybir
import concourse.tile as tile
from concourse import bass_utils

N_CORES = 8
B, D = 8192, 4096
BS = B // N_CORES          # 1024 samples per core
P = 128                    # SBUF partitions
T = BS // P                # 8 tiles of 128 samples per core
EPS = 1e-6

f32 = mybir.dt.float32
Alu = mybir.AluOpType
Act = mybir.ActivationFunctionType

_CACHE = {}


def _build_program():
    nc = bacc.Bacc(
        "TRN2",
        target_bir_lowering=False,
        debug=False,
        num_devices=N_CORES,
    )

    samples = nc.dram_tensor("samples", [BS, 3, D], f32, kind="ExternalInput")
    labels = nc.dram_tensor("labels", [1], f32, kind="ExternalInput")
    dv1 = nc.dram_tensor("dv1", [BS], f32, kind="ExternalInput")
    dv2 = nc.dram_tensor("dv2", [BS], f32, kind="ExternalInput")
    out = nc.dram_tensor("out", [1, 1], f32, kind="ExternalOutput")

    with tile.TileContext(nc) as tc:
        with (
            tc.tile_pool(name="data", bufs=2) as data_pool,
            tc.tile_pool(name="junk", bufs=2) as junk_pool,
            tc.tile_pool(name="stats", bufs=1) as stats_pool,
            tc.tile_pool(name="psum", bufs=1, space="PSUM") as psum_pool,
            tc.tile_pool(name="dram", bufs=1, space="DRAM") as dram_pool,
        ):
            # Per-sample statistics, one column per 128-sample tile.
            qd_s = stats_pool.tile([P, T], f32, tag="qd")
            ad_s = stats_pool.tile([P, T], f32, tag="ad")
            qq_s = stats_pool.tile([P, T], f32, tag="qq")
            aa_s = stats_pool.tile([P, T], f32, tag="aa")
            dd_s = stats_pool.tile([P, T], f32, tag="dd")

            for t in range(T):
                # One contiguous 6 MB DMA: 128 samples x (q,a,d) x 4096 f32.
                dat = data_pool.tile([P, 3, D], f32, tag="dat")
                nc.sync.dma_start(dat[:], samples[bass.ts(t, P), :, :])
                q = dat[:, 0, :]
                a = dat[:, 1, :]
                d = dat[:, 2, :]

                # DVE: fused product + per-partition accumulate
                # (scalar_tensor_tensor; accum_out must be a standalone
                # tile — strided accum destinations crash the HW).
                for src0, src1, dst, atag in (
                    (q, d, qd_s, "qd1"),
                    (a, d, ad_s, "ad1"),
                    (d, d, dd_s, "dd1"),
                ):
                    jd = junk_pool.tile([P, D], f32, tag="junk_dve")
                    acc = junk_pool.tile([P, 1], f32, tag=atag)
                    nc.vector.scalar_tensor_tensor(
                        out=jd[:], in0=src0, scalar=1.0, in1=src1,
                        op0=Alu.mult, op1=Alu.mult, accum_out=acc[:],
                    )
                    nc.vector.tensor_copy(dst[:, t : t + 1], acc[:])

                # ACT: square + accumulate for the q/a norms.
                for src0, dst, atag in ((q, qq_s, "qq1"), (a, aa_s, "aa1")):
                    ja = junk_pool.tile([P, D], f32, tag="junk_act")
                    acc = junk_pool.tile([P, 1], f32, tag=atag)
                    nc.scalar.activation(
                        out=ja[:], in_=src0, func=Act.Square, accum_out=acc[:],
                    )
                    nc.vector.tensor_copy(dst[:, t : t + 1], acc[:])

            # cos = dot / max(sqrt(n1*n2), EPS), all on [128, T] stats.
            small = stats_pool.tile([P, T], f32, tag="small0")
            inv1 = stats_pool.tile([P, T], f32, tag="inv1")
            nc.vector.tensor_mul(small[:], qq_s[:], dd_s[:])
            nc.scalar.sqrt(small[:], small[:])
            nc.vector.tensor_scalar_max(small[:], small[:], EPS)
            nc.vector.reciprocal(inv1[:], small[:])

            small2 = stats_pool.tile([P, T], f32, tag="small2")
            inv2 = stats_pool.tile([P, T], f32, tag="inv2")
            nc.vector.tensor_mul(small2[:], aa_s[:], dd_s[:])
            nc.scalar.sqrt(small2[:], small2[:])
            nc.vector.tensor_scalar_max(small2[:], small2[:], EPS)
            nc.vector.reciprocal(inv2[:], small2[:])

            cos1 = stats_pool.tile([P, T], f32, tag="cos1")
            cos2 = stats_pool.tile([P, T], f32, tag="cos2")
            nc.vector.tensor_mul(cos1[:], qd_s[:], inv1[:])
            nc.vector.tensor_mul(cos2[:], ad_s[:], inv2[:])

            # Weight by D_v1/D_v2 (laid out [p, t] to match the stats tiles).
            dv1_t = stats_pool.tile([P, T], f32, tag="dv1")
            dv2_t = stats_pool.tile([P, T], f32, tag="dv2")
            nc.sync.dma_start(dv1_t[:], dv1[:].rearrange("(n p) -> p n", p=P))
            nc.sync.dma_start(dv2_t[:], dv2[:].rearrange("(n p) -> p n", p=P))

            contrib = stats_pool.tile([P, T], f32, tag="contrib")
            contrib2 = stats_pool.tile([P, T], f32, tag="contrib2")
            nc.vector.tensor_mul(contrib[:], cos1[:], dv1_t[:])
            nc.vector.tensor_mul(contrib2[:], cos2[:], dv2_t[:])
            nc.vector.tensor_add(contrib[:], contrib[:], contrib2[:])

            row_sum = stats_pool.tile([P, 1], f32, tag="row_sum")
            nc.vector.reduce_sum(row_sum[:], contrib[:], axis=mybir.AxisListType.X)

            # Partition reduction via PE: [1,1] = row_sum^T @ ones.
            ones = stats_pool.tile([P, 1], f32, tag="ones")
            nc.gpsimd.memset(ones[:], 1.0)
            psum_t = psum_pool.tile([1, 1], f32, tag="psum_s")
            nc.tensor.matmul(psum_t[:], row_sum[:], ones[:], start=True, stop=True)

            # Stage the partial score, AllReduce across the 8 cores.
            partial = stats_pool.tile([1, 8], f32, tag="partial")
            nc.gpsimd.memset(partial[:], 0.0)
            nc.vector.tensor_copy(partial[0:1, 0:1], psum_t[:])

            cc_in = dram_pool.tile([1, 8], f32, tag="cc_in")
            cc_out = dram_pool.tile([1, 8], f32, tag="cc_out")
            nc.sync.dma_start(cc_in[:], partial[:])
            nc.gpsimd.collective_compute(
                "AllReduce",
                Alu.add,
                replica_groups=[list(range(N_CORES))],
                ins=[cc_in[:].opt()],
                outs=[cc_out[:].opt()],
            )
            red = stats_pool.tile([1, 8], f32, tag="red")
            nc.sync.dma_start(red[:], cc_out[:])
            s = red[0:1, 0:1]

            # BCE with logits: max(s,0) - s*y + softplus(-|s|), on [1,1].
            ltile = stats_pool.tile([1, 1], f32, tag="ltile")
            nc.sync.dma_start(ltile[:], labels[None, :])

            relu_t = stats_pool.tile([1, 1], f32, tag="relu_t")
            abs_t = stats_pool.tile([1, 1], f32, tag="abs_t")
            exp_t = stats_pool.tile([1, 1], f32, tag="exp_t")
            sp_t = stats_pool.tile([1, 1], f32, tag="sp_t")
            xy_t = stats_pool.tile([1, 1], f32, tag="xy_t")
            bce_t = stats_pool.tile([1, 1], f32, tag="bce_t")
            nc.scalar.activation(relu_t[:], s, Act.Relu)
            nc.scalar.activation(abs_t[:], s, Act.Abs)
            # softplus(-|s|) = ln(1 + exp(-|s|)); Softplus has no HW table.
            nc.scalar.activation(exp_t[:], abs_t[:], Act.Exp, scale=-1.0)
            nc.scalar.activation(sp_t[:], exp_t[:], Act.Ln, bias=1.0)
            nc.vector.tensor_mul(xy_t[:], s, ltile[:])
            nc.vector.tensor_sub(bce_t[:], relu_t[:], xy_t[:])
            nc.vector.tensor_add(bce_t[:], bce_t[:], sp_t[:])

            nc.sync.dma_start(out[:], bce_t[:])

    nc.compile()
    return nc


def _get_program():
    if "nc" not in _CACHE:
        _CACHE["nc"] = _build_program()
    return _CACHE["nc"]


def kernel(samples, labels, D_v1, D_v2):
    samples = np.asarray(samples, dtype=np.float32)
    labels = np.asarray(labels, dtype=np.float32)
    D_v1 = np.asarray(D_v1, dtype=np.float32)
    D_v2 = np.asarray(D_v2, dtype=np.float32)
    assert samples.shape == (B, 3, D), samples.shape

    nc = _get_program()

    in_maps = []
    for c in range(N_CORES):
        sl = slice(c * BS, (c + 1) * BS)
        in_maps.append(
            {
                "samples": np.ascontiguousarray(samples[sl]),
                "labels": labels,
                "dv1": np.ascontiguousarray(D_v1[sl]),
                "dv2": np.ascontiguousarray(D_v2[sl]),
            }
        )

    res = bass_utils.run_bass_kernel_spmd(nc, in_maps, core_ids=list(range(N_CORES)))
    _CACHE["last_results"] = res
    return np.asarray(res.results[0]["out"], dtype=np.float32).reshape(())
